# revision 12
# baseline (speedup 1.0000x reference)
"""Mamba (ArceeMamba) block on 8 TRN2 NeuronCores.

Sharding: core c -> (batch b = c//4, d_inner shard s = c%4 of 512 channels).
Per core, the full sequence (L=4096) is processed in chunks of T along time:
  in_proj (PE, bf16) -> causal depthwise conv (DVE) -> silu (ACT)
  -> x_proj partial (PE) -> AllReduce (96,T) over the 4 shard cores
  -> dt_proj (PE) + softplus (ACT)
  -> selective scan via DVE tensor_tensor_scan over (d,n) slabs, time on free axis
  -> y = sum_n C*h (DVE) + D*x_c, gate with silu(z), out_proj partial (PE)
Partial out_proj results (d_model x L) are summed on the host across the 4
shard cores of each batch.
"""

import sys

for _p in ("/opt/trn_rl_repo", "/root/.axon_site/_ro/trn_rl_repo"):
    if _p not in sys.path:
        sys.path.insert(0, _p)

import numpy as np
import ml_dtypes

import concourse.bass as bass
from concourse import bacc
import concourse.mybir as mybir
import concourse.tile as tile
from concourse.bass import ts, ds
from concourse.bass_utils import run_bass_kernel_spmd

FP32 = mybir.dt.float32
BF16 = mybir.dt.bfloat16
AF = mybir.ActivationFunctionType
OP = mybir.AluOpType

B, L, DM = 2, 4096, 1024
DI, N, DC, R = 2048, 16, 4, 64
NCORE = 8
NSH = 4                 # d_inner shards (cores per batch)
DS = DI // NSH          # 512 channels per core
DBLK = DS // 128        # 4 d-blocks of 128 partitions
T = 256                 # time chunk
NCHUNK = L // T
NSLAB = 4               # n-states per slab
SLABS = N // NSLAB      # slabs per d-block
LQ = L // NSH           # per-core time slice of hidden_states (AllGather)
PQ = 128 // NSH         # output partition rows per core (ReduceScatter)

REPLICA_GROUPS = [[0, 1, 2, 3], [4, 5, 6, 7]]

BF = ml_dtypes.bfloat16


def build_nc():
    nc = bacc.Bacc()

    hidT = nc.declare_dram_parameter("hidT", [128, 8, LQ], BF16, isOutput=False)
    wxzT = nc.declare_dram_parameter("wxzT", [128, 8, 2 * DS], BF16, isOutput=False)
    wxpT = nc.declare_dram_parameter("wxpT", [128, DBLK, 96], BF16, isOutput=False)
    wdtT = nc.declare_dram_parameter("wdtT", [64, DS], BF16, isOutput=False)
    wopT = nc.declare_dram_parameter("wopT", [128, DBLK, DM], BF16, isOutput=False)
    # packed small constants: [convw(16) | convb(4) | dtb(4) | dsk(4) | acol(64)]
    consts = nc.declare_dram_parameter("consts", [128, 92], FP32, isOutput=False)
    out = nc.declare_dram_parameter("out", [PQ, 8, L], BF16, isOutput=True)

    from contextlib import ExitStack

    with tile.TileContext(nc) as tc:
        with ExitStack() as st:
            def pool(name, bufs, space="SBUF"):
                return st.enter_context(
                    tc.tile_pool(name=name, bufs=bufs, space=space)
                )

            wp = pool("wp", 1)
            hidp = pool("hidp", 2)
            xp = pool("xp", 2)
            cvp = pool("cvp", 2)
            xcbfp = pool("xcbfp", 2)
            xcfp = pool("xcfp", 2)
            zsp = pool("zsp", 2)
            dtp = pool("dtp", 2)
            dtxp = pool("dtxp", 2)
            dblp = pool("dblp", 2)
            bcp = pool("bcp", 1)
            ap_ = pool("ap_", 2)
            bxp = pool("bxp", 2)
            hp = pool("hp", 2)
            hcp = pool("hcp", 2)
            yrp = pool("yrp", 2)
            yp = pool("yp", 2)
            gp = pool("gp", 2)
            op_ = pool("op_", 2)
            mmp = pool("mmp", 3, "PSUM")
            psml = pool("psml", 2, "PSUM")
            pout = pool("pout", 3, "PSUM")
            drp = pool("drp", 2, "DRAM")
            gdp = pool("gdp", 1, "DRAM")
            # ---- AllGather the time-sharded hidden_states input ----
            # (collectives cannot touch IO tensors directly; stage via
            # an internal DRAM tile)
            hid_stage = gdp.tile([128, 8, LQ], BF16, tag="hidstage")
            nc.sync.dma_start(hid_stage[:], hidT[:])
            hid_all = gdp.tile([NSH, 128, 8, LQ], BF16, tag="hidall", addr_space="Shared")
            nc.gpsimd.collective_compute(
                "AllGather",
                OP.bypass,
                replica_groups=REPLICA_GROUPS,
                ins=[hid_stage[:]],
                outs=[hid_all[:]],
            )
            # partial out_proj accumulator (reduced across shard cores at end)
            pout_dram = gdp.tile([128, 8, L], BF16, tag="poutdram")
            # ---- resident weights ----
            wxz_sb = wp.tile([128, 8, 2 * DS], BF16, tag="wxz")
            nc.sync.dma_start(wxz_sb[:], wxzT[:])
            wxp_sb = wp.tile([128, DBLK, 96], BF16, tag="wxp")
            nc.sync.dma_start(wxp_sb[:], wxpT[:])
            wdt_sb = wp.tile([64, DS], BF16, tag="wdt")
            nc.sync.dma_start(wdt_sb[:], wdtT[:])
            wop_sb = wp.tile([128, DBLK, DM], BF16, tag="wop")
            nc.sync.dma_start(wop_sb[:], wopT[:])
            consts_sb = wp.tile([128, 92], FP32, tag="consts")
            nc.sync.dma_start(consts_sb[:], consts[:])
            convw_sb = consts_sb[:, 0:16].rearrange("p (db k) -> p db k", db=DBLK)
            convb_sb = consts_sb[:, 16:20]
            dtb_sb = consts_sb[:, 20:24]
            d_sb = consts_sb[:, 24:28]
            a_sb = consts_sb[:, 28:92].rearrange("p (db n) -> p db n", db=DBLK)

            carry = wp.tile([128, DS // 128 * N], FP32, tag="carry")  # (128, 64)
            nc.vector.memset(carry[:], 0.0)
            halo = wp.tile([128, DBLK, DC - 1], FP32, tag="halo")
            nc.vector.memset(halo[:], 0.0)

            for c in range(NCHUNK):
                t0 = c * T

                g, off = divmod(t0, LQ)
                hid = hidp.tile([128, 8, T], BF16, tag="hid")
                nc.sync.dma_start(hid[:], hid_all[g, :, :, ds(off, T)])

                # ---- in_proj: x (m 0..3) and z (m 4..7) ----
                x_sb = xp.tile([128, DBLK, T + DC - 1], FP32, tag="x")
                zs_sb = zsp.tile([128, DBLK, T], FP32, tag="zs")
                # halo columns from previous chunk
                nc.vector.tensor_copy(x_sb[:, :, 0 : DC - 1], halo[:])
                for m in range(8):
                    px = mmp.tile([128, T], FP32, tag="mm")
                    for k in range(8):
                        nc.tensor.matmul(
                            px[:],
                            wxz_sb[:, k, ts(m, 128)],
                            hid[:, k, :],
                            start=(k == 0),
                            stop=(k == 7),
                        )
                    if m < 4:
                        nc.scalar.activation(
                            x_sb[:, m, DC - 1 : DC - 1 + T], px[:], AF.Copy
                        )
                    else:
                        nc.scalar.activation(zs_sb[:, m - 4, :], px[:], AF.Silu)
                # save halo for next chunk
                nc.vector.tensor_copy(halo[:], x_sb[:, :, T : T + DC - 1])

                # ---- causal depthwise conv ----
                cv = cvp.tile([128, DBLK, T], FP32, tag="cv")
                for db in range(DBLK):
                    nc.vector.tensor_scalar(
                        cv[:, db, :],
                        x_sb[:, db, DC - 1 : DC - 1 + T],
                        convw_sb[:, db, DC - 1 : DC],
                        convb_sb[:, db : db + 1],
                        op0=OP.mult,
                        op1=OP.add,
                    )
                    for k in range(DC - 1):
                        nc.vector.scalar_tensor_tensor(
                            cv[:, db, :],
                            x_sb[:, db, k : k + T],
                            convw_sb[:, db, k : k + 1],
                            cv[:, db, :],
                            op0=OP.mult,
                            op1=OP.add,
                        )

                xc_bf = xcbfp.tile([128, DBLK, T], BF16, tag="xcbf")
                nc.scalar.activation(xc_bf[:], cv[:], AF.Silu)
                xc_f = xcfp.tile([128, DBLK, T], FP32, tag="xcf")
                nc.scalar.activation(xc_f[:], cv[:], AF.Silu)

                # ---- x_proj partial + AllReduce ----
                pdbl = psml.tile([96, T], FP32, tag="pdbl")
                for db in range(DBLK):
                    nc.tensor.matmul(
                        pdbl[:],
                        wxp_sb[:, db, :],
                        xc_bf[:, db, :],
                        start=(db == 0),
                        stop=(db == DBLK - 1),
                    )
                dbl_sb = dblp.tile([96, T], FP32, tag="dbl")
                nc.scalar.activation(dbl_sb[:], pdbl[:], AF.Copy)

                cc_in = drp.tile([96, T], FP32, tag="ccin")
                cc_out = drp.tile([96, T], FP32, tag="ccout", addr_space="Shared")
                nc.sync.dma_start(cc_in[:], dbl_sb[:])
                nc.gpsimd.collective_compute(
                    "AllReduce",
                    OP.add,
                    replica_groups=REPLICA_GROUPS,
                    ins=[cc_in[:]],
                    outs=[cc_out[:]],
                )
                dtlow_f = dblp.tile([64, T], FP32, tag="dtlowf")
                nc.sync.dma_start(dtlow_f[:], cc_out[0:64, :])
                dtlow_bf = dblp.tile([64, T], BF16, tag="dtlow")
                nc.vector.tensor_copy(dtlow_bf[:], dtlow_f[:])

                # ---- broadcast B and C rows to 128 partitions (stride-0 DMA) ----
                # cast rows to bf16 first so the big dBx/hC tensor_tensor ops
                # run in the DVE 16-bit 2x mode
                bcst = dblp.tile([32, T], FP32, tag="bcst")
                nc.sync.dma_start(bcst[:], cc_out[64:96, :])
                bcst_bf = dblp.tile([32, T], BF16, tag="bcstbf")
                nc.vector.tensor_copy(bcst_bf[:], bcst[:])
                bc_dram = drp.tile([32, T], BF16, tag="bcdram")
                nc.sync.dma_start(bc_dram[:], bcst_bf[:])
                bc_all = bcp.tile([128, 2 * N, T], BF16, tag="bcall")
                nc.sync.dma_start(
                    bc_all[:],
                    bc_dram[:, :].rearrange("n t -> () n t").broadcast_to(
                        [128, 2 * N, T]
                    ),
                )
                b_all = bc_all[:, 0:N, :]
                c_all = bc_all[:, N : 2 * N, :]

                # ---- dt_proj + softplus ----
                dt_sb = dtp.tile([128, DBLK, T], FP32, tag="dt")
                for m in range(DBLK):
                    pdt = mmp.tile([128, T], FP32, tag="mm")
                    nc.tensor.matmul(
                        pdt[:], wdt_sb[:, ts(m, 128)], dtlow_bf[:], start=True, stop=True
                    )
                    # softplus(v + dtb) as ln(exp(v + dtb) + 1)
                    edt = dblp.tile([128, T], FP32, tag="edt")
                    nc.scalar.activation(
                        edt[:], pdt[:], AF.Exp, bias=dtb_sb[:, m : m + 1]
                    )
                    nc.scalar.activation(dt_sb[:, m, :], edt[:], AF.Ln, bias=1.0)

                dtx = dtxp.tile([128, DBLK, T], BF16, tag="dtx")
                nc.vector.tensor_tensor(dtx[:], dt_sb[:], xc_f[:], op=OP.mult)

                # ---- selective scan over (db, n) slabs ----
                y_sb = yp.tile([128, DBLK, T], FP32, tag="y")
                for db in range(DBLK):
                    for s in range(SLABS):
                        n0 = s * NSLAB
                        da = ap_.tile([128, NSLAB, T], FP32, tag="da")
                        for j in range(NSLAB):
                            nc.scalar.activation(
                                da[:, j, :],
                                dt_sb[:, db, :],
                                AF.Exp,
                                scale=a_sb[:, db, n0 + j : n0 + j + 1],
                            )
                        dbx = bxp.tile([128, NSLAB, T], BF16, tag="dbx")
                        for j in range(NSLAB):
                            nc.vector.tensor_tensor(
                                dbx[:, j, :], dtx[:, db, :], b_all[:, n0 + j, :],
                                op=OP.mult,
                            )
                        h = hp.tile([128, NSLAB, T], BF16, tag="h")
                        for j in range(NSLAB):
                            ci = db * N + n0 + j
                            nc.vector.tensor_tensor_scan(
                                h[:, j, :],
                                da[:, j, :],
                                dbx[:, j, :],
                                initial=carry[:, ci : ci + 1],
                                op0=OP.mult,
                                op1=OP.add,
                            )
                        nc.vector.tensor_copy(
                            carry[:, db * N + n0 : db * N + n0 + NSLAB],
                            h[:, :, T - 1],
                        )
                        hc = hcp.tile([128, NSLAB, T], BF16, tag="hc")
                        nc.vector.tensor_tensor(
                            hc[:], h[:], c_all[:, n0 : n0 + NSLAB, :], op=OP.mult
                        )
                        if s == 0:
                            nc.vector.tensor_reduce(
                                y_sb[:, db, :],
                                hc.rearrange("p n t -> p t n"),
                                axis=mybir.AxisListType.X,
                                op=OP.add,
                            )
                        else:
                            yr = yrp.tile([128, T], FP32, tag="yr")
                            nc.vector.tensor_reduce(
                                yr[:],
                                hc.rearrange("p n t -> p t n"),
                                axis=mybir.AxisListType.X,
                                op=OP.add,
                            )
                            nc.vector.tensor_tensor(
                                y_sb[:, db, :], y_sb[:, db, :], yr[:], op=OP.add
                            )

                # ---- D skip term, gate, out_proj ----
                for db in range(DBLK):
                    nc.vector.scalar_tensor_tensor(
                        y_sb[:, db, :],
                        xc_f[:, db, :],
                        d_sb[:, db : db + 1],
                        y_sb[:, db, :],
                        op0=OP.mult,
                        op1=OP.add,
                    )
                gated = gp.tile([128, DBLK, T], BF16, tag="gated")
                nc.vector.tensor_tensor(gated[:], y_sb[:], zs_sb[:], op=OP.mult)

                out_sb = op_.tile([128, 8, T], BF16, tag="out")
                for m in range(8):
                    po = pout.tile([128, T], FP32, tag="po")
                    for db in range(DBLK):
                        nc.tensor.matmul(
                            po[:],
                            wop_sb[:, db, ts(m, 128)],
                            gated[:, db, :],
                            start=(db == 0),
                            stop=(db == DBLK - 1),
                        )
                    nc.scalar.activation(out_sb[:, m, :], po[:], AF.Copy)
                nc.sync.dma_start(pout_dram[:, :, ds(t0, T)], out_sb[:])

            # ---- ReduceScatter partial outputs across the 4 shard cores ----
            rs_out = gdp.tile([PQ, 8, L], BF16, tag="rsout", addr_space="Shared")
            nc.gpsimd.collective_compute(
                "ReduceScatter",
                OP.add,
                replica_groups=REPLICA_GROUPS,
                ins=[pout_dram[:]],
                outs=[rs_out[:]],
            )
            nc.sync.dma_start(out[:], rs_out[:])

    nc.finalize()
    return nc


_NC_CACHE = {}


def get_nc():
    if "nc" not in _NC_CACHE:
        _NC_CACHE["nc"] = build_nc()
    return _NC_CACHE["nc"]


def make_in_maps(inputs):
    hs = np.asarray(inputs["hidden_states"], np.float32)
    w_in = np.asarray(inputs["in_proj_w"], np.float32)
    conv_w = np.asarray(inputs["conv_w"], np.float32)
    conv_b = np.asarray(inputs["conv_b"], np.float32)
    w_xp = np.asarray(inputs["x_proj_w"], np.float32)
    w_dt = np.asarray(inputs["dt_proj_w"], np.float32)
    b_dt = np.asarray(inputs["dt_proj_b"], np.float32)
    a_log = np.asarray(inputs["A_log"], np.float32)
    d_skip = np.asarray(inputs["D"], np.float32)
    w_op = np.asarray(inputs["out_proj_w"], np.float32)

    a_full = -np.exp(a_log)  # (DI, N)

    in_maps = []
    for c in range(NCORE):
        b, s = divmod(c, NSH)
        d0 = s * DS
        sl = slice(d0, d0 + DS)

        hidT = np.ascontiguousarray(
            hs[b].T.reshape(8, 128, L).transpose(1, 0, 2)[:, :, s * LQ : (s + 1) * LQ]
        ).astype(BF)

        w_cat = np.concatenate([w_in[sl], w_in[DI + d0 : DI + d0 + DS]], 0)  # (1024, DM)
        wxzT = np.ascontiguousarray(
            w_cat.T.reshape(8, 128, 2 * DS).transpose(1, 0, 2)
        ).astype(BF)

        wxpT = np.ascontiguousarray(
            w_xp[:, sl].T.reshape(DBLK, 128, 96).transpose(1, 0, 2)
        ).astype(BF)
        wdtT = np.ascontiguousarray(w_dt[sl].T).astype(BF)  # (64, 512)
        wopT = np.ascontiguousarray(
            w_op[:, sl].T.reshape(DBLK, 128, DM).transpose(1, 0, 2)
        ).astype(BF)

        convw = np.ascontiguousarray(
            conv_w[sl].reshape(DBLK, 128, DC).transpose(1, 0, 2), np.float32
        )
        convb = np.ascontiguousarray(
            conv_b[sl].reshape(DBLK, 128).T, np.float32
        )
        dtb = np.ascontiguousarray(b_dt[sl].reshape(DBLK, 128).T, np.float32)
        dsk = np.ascontiguousarray(d_skip[sl].reshape(DBLK, 128).T, np.float32)
        acol = np.ascontiguousarray(
            a_full[sl].reshape(DBLK, 128, N).transpose(1, 0, 2), np.float32
        )

        consts = np.concatenate(
            [convw.reshape(128, 16), convb, dtb, dsk, acol.reshape(128, 64)], axis=1
        ).astype(np.float32)

        in_maps.append(
            dict(
                hidT=hidT, wxzT=wxzT, wxpT=wxpT, wdtT=wdtT, wopT=wopT,
                consts=np.ascontiguousarray(consts),
            )
        )
    return in_maps


def gather_output(results):
    outs = []
    for b in range(B):
        # core (b, s) holds partition rows [s*PQ, (s+1)*PQ) of the reduced
        # (128, 8, L) output (ReduceScatter splits the flat buffer into 4
        # contiguous chunks, rank s gets chunk s)
        acc = np.concatenate(
            [np.asarray(results[b * NSH + s]["out"], np.float32) for s in range(NSH)],
            axis=0,
        )
        # (128, 8, L) [p, m, t] -> (1024, L) -> (L, 1024)
        full_t = acc.transpose(1, 0, 2).reshape(DM, L)
        outs.append(full_t.T)
    return np.stack(outs).astype(np.float32)


def run_on_hw(inputs, trace=False, **kwargs):
    nc = get_nc()
    in_maps = make_in_maps(inputs)
    res = run_bass_kernel_spmd(
        nc, in_maps, core_ids=list(range(NCORE)), trace=trace, **kwargs
    )
    return res


def kernel(**inputs):
    res = run_on_hw(inputs, trace=False)
    return gather_output(res.results)



# revision 13
# speedup vs baseline: 1.5341x; 1.5341x over previous
"""Mamba (ArceeMamba) block on 8 TRN2 NeuronCores.

Sharding: core c -> (batch b = c//4, d_inner shard s = c%4 of 512 channels).
Per core, the full sequence (L=4096) is processed in chunks of T along time:
  in_proj (PE, bf16) -> causal depthwise conv (DVE) -> silu (ACT)
  -> x_proj partial (PE) -> AllReduce (96,T) over the 4 shard cores
  -> dt_proj (PE) + softplus (ACT)
  -> selective scan via DVE tensor_tensor_scan over (d,n) slabs, time on free axis
  -> y = sum_n C*h (DVE) + D*x_c, gate with silu(z), out_proj partial (PE)
Partial out_proj results (d_model x L) are summed on the host across the 4
shard cores of each batch.
"""

import sys

for _p in ("/opt/trn_rl_repo", "/root/.axon_site/_ro/trn_rl_repo"):
    if _p not in sys.path:
        sys.path.insert(0, _p)

import numpy as np
import ml_dtypes

import concourse.bass as bass
from concourse import bacc
import concourse.mybir as mybir
import concourse.tile as tile
from concourse.bass import ts, ds
from concourse.bass_utils import run_bass_kernel_spmd

FP32 = mybir.dt.float32
BF16 = mybir.dt.bfloat16
AF = mybir.ActivationFunctionType
OP = mybir.AluOpType

B, L, DM = 2, 4096, 1024
DI, N, DC, R = 2048, 16, 4, 64
NCORE = 8
NSH = 4                 # d_inner shards (cores per batch)
DS = DI // NSH          # 512 channels per core
DBLK = DS // 128        # 4 d-blocks of 128 partitions
T = 256                 # time chunk
NCHUNK = L // T
NSLAB = 4               # n-states per slab
SLABS = N // NSLAB      # slabs per d-block
LQ = L // NSH           # per-core time slice of hidden_states (AllGather)
PQ = 128 // NSH         # output partition rows per core (ReduceScatter)

REPLICA_GROUPS = [[0, 1, 2, 3], [4, 5, 6, 7]]

BF = ml_dtypes.bfloat16


def build_nc():
    nc = bacc.Bacc()

    hidT = nc.declare_dram_parameter("hidT", [128, 8, LQ], BF16, isOutput=False)
    wxzT = nc.declare_dram_parameter("wxzT", [128, 8, 2 * DS], BF16, isOutput=False)
    wxpT = nc.declare_dram_parameter("wxpT", [128, DBLK, 96], BF16, isOutput=False)
    wdtT = nc.declare_dram_parameter("wdtT", [64, DS], BF16, isOutput=False)
    wopT = nc.declare_dram_parameter("wopT", [128, DBLK, DM], BF16, isOutput=False)
    # packed small constants: [convw(16) | convb(4) | dtb(4) | dsk(4) | acol(64)]
    consts = nc.declare_dram_parameter("consts", [128, 92], FP32, isOutput=False)
    out = nc.declare_dram_parameter("out", [PQ, 8, L], BF16, isOutput=True)

    from contextlib import ExitStack

    with tile.TileContext(nc) as tc:
        with ExitStack() as st:
            def pool(name, bufs, space="SBUF"):
                return st.enter_context(
                    tc.tile_pool(name=name, bufs=bufs, space=space)
                )

            wp = pool("wp", 1)
            hidp = pool("hidp", 2)
            xp = pool("xp", 2)
            cvp = pool("cvp", 2)
            xcbfp = pool("xcbfp", 2)
            xcfp = pool("xcfp", 2)
            zsp = pool("zsp", 2)
            dtp = pool("dtp", 2)
            dtxp = pool("dtxp", 2)
            dblp = pool("dblp", 2)
            bcp = pool("bcp", 1)
            ap_ = pool("ap_", 2)
            bxp = pool("bxp", 2)
            hp = pool("hp", 2)
            hcp = pool("hcp", 2)
            yrp = pool("yrp", 2)
            yp = pool("yp", 2)
            gp = pool("gp", 2)
            op_ = pool("op_", 2)
            mmp = pool("mmp", 3, "PSUM")
            psml = pool("psml", 2, "PSUM")
            pout = pool("pout", 3, "PSUM")
            drp = pool("drp", 2, "DRAM")
            gdp = pool("gdp", 1, "DRAM")
            # ---- AllGather the time-sharded hidden_states input ----
            # (collectives cannot touch IO tensors directly; stage via
            # an internal DRAM tile)
            hid_stage = gdp.tile([128, 8, LQ], BF16, tag="hidstage")
            nc.sync.dma_start(hid_stage[:], hidT[:])
            hid_all = gdp.tile([NSH, 128, 8, LQ], BF16, tag="hidall")
            nc.gpsimd.collective_compute(
                "AllGather",
                OP.bypass,
                replica_groups=REPLICA_GROUPS,
                ins=[hid_stage[:]],
                outs=[hid_all[:]],
            )
            # partial out_proj accumulator (reduced across shard cores at end)
            pout_dram = gdp.tile([128, 8, L], BF16, tag="poutdram")
            # ---- resident weights ----
            wxz_sb = wp.tile([128, 8, 2 * DS], BF16, tag="wxz")
            nc.sync.dma_start(wxz_sb[:], wxzT[:])
            wxp_sb = wp.tile([128, DBLK, 96], BF16, tag="wxp")
            nc.sync.dma_start(wxp_sb[:], wxpT[:])
            wdt_sb = wp.tile([64, DS], BF16, tag="wdt")
            nc.sync.dma_start(wdt_sb[:], wdtT[:])
            wop_sb = wp.tile([128, DBLK, DM], BF16, tag="wop")
            nc.sync.dma_start(wop_sb[:], wopT[:])
            consts_sb = wp.tile([128, 92], FP32, tag="consts")
            nc.sync.dma_start(consts_sb[:], consts[:])
            convw_sb = consts_sb[:, 0:16].rearrange("p (db k) -> p db k", db=DBLK)
            convb_sb = consts_sb[:, 16:20]
            dtb_sb = consts_sb[:, 20:24]
            d_sb = consts_sb[:, 24:28]
            a_sb = consts_sb[:, 28:92].rearrange("p (db n) -> p db n", db=DBLK)

            carry = wp.tile([128, DS // 128 * N], FP32, tag="carry")  # (128, 64)
            nc.vector.memset(carry[:], 0.0)
            halo = wp.tile([128, DBLK, DC - 1], FP32, tag="halo")
            nc.vector.memset(halo[:], 0.0)

            for c in range(NCHUNK):
                t0 = c * T

                g, off = divmod(t0, LQ)
                hid = hidp.tile([128, 8, T], BF16, tag="hid")
                nc.sync.dma_start(hid[:], hid_all[g, :, :, ds(off, T)])

                # ---- in_proj: x (m 0..3) and z (m 4..7) ----
                x_sb = xp.tile([128, DBLK, T + DC - 1], FP32, tag="x")
                zs_sb = zsp.tile([128, DBLK, T], FP32, tag="zs")
                # halo columns from previous chunk
                nc.vector.tensor_copy(x_sb[:, :, 0 : DC - 1], halo[:])
                for m in range(8):
                    px = mmp.tile([128, T], FP32, tag="mm")
                    for k in range(8):
                        nc.tensor.matmul(
                            px[:],
                            wxz_sb[:, k, ts(m, 128)],
                            hid[:, k, :],
                            start=(k == 0),
                            stop=(k == 7),
                        )
                    if m < 4:
                        nc.scalar.activation(
                            x_sb[:, m, DC - 1 : DC - 1 + T], px[:], AF.Copy
                        )
                    else:
                        nc.scalar.activation(zs_sb[:, m - 4, :], px[:], AF.Silu)
                # save halo for next chunk
                nc.vector.tensor_copy(halo[:], x_sb[:, :, T : T + DC - 1])

                # ---- causal depthwise conv ----
                cv = cvp.tile([128, DBLK, T], FP32, tag="cv")
                for db in range(DBLK):
                    nc.vector.tensor_scalar(
                        cv[:, db, :],
                        x_sb[:, db, DC - 1 : DC - 1 + T],
                        convw_sb[:, db, DC - 1 : DC],
                        convb_sb[:, db : db + 1],
                        op0=OP.mult,
                        op1=OP.add,
                    )
                    for k in range(DC - 1):
                        nc.vector.scalar_tensor_tensor(
                            cv[:, db, :],
                            x_sb[:, db, k : k + T],
                            convw_sb[:, db, k : k + 1],
                            cv[:, db, :],
                            op0=OP.mult,
                            op1=OP.add,
                        )

                xc_bf = xcbfp.tile([128, DBLK, T], BF16, tag="xcbf")
                nc.scalar.activation(xc_bf[:], cv[:], AF.Silu)
                xc_f = xcfp.tile([128, DBLK, T], FP32, tag="xcf")
                nc.scalar.activation(xc_f[:], cv[:], AF.Silu)

                # ---- x_proj partial + AllReduce ----
                pdbl = psml.tile([96, T], FP32, tag="pdbl")
                for db in range(DBLK):
                    nc.tensor.matmul(
                        pdbl[:],
                        wxp_sb[:, db, :],
                        xc_bf[:, db, :],
                        start=(db == 0),
                        stop=(db == DBLK - 1),
                    )
                dbl_sb = dblp.tile([96, T], FP32, tag="dbl")
                nc.scalar.activation(dbl_sb[:], pdbl[:], AF.Copy)

                cc_in = drp.tile([96, T], FP32, tag="ccin")
                cc_out = drp.tile([96, T], FP32, tag="ccout")
                nc.sync.dma_start(cc_in[:], dbl_sb[:])
                nc.gpsimd.collective_compute(
                    "AllReduce",
                    OP.add,
                    replica_groups=REPLICA_GROUPS,
                    ins=[cc_in[:]],
                    outs=[cc_out[:]],
                )
                dtlow_f = dblp.tile([64, T], FP32, tag="dtlowf")
                nc.sync.dma_start(dtlow_f[:], cc_out[0:64, :])
                dtlow_bf = dblp.tile([64, T], BF16, tag="dtlow")
                nc.vector.tensor_copy(dtlow_bf[:], dtlow_f[:])

                # ---- broadcast B and C rows to 128 partitions (stride-0 DMA) ----
                # cast rows to bf16 first so the big dBx/hC tensor_tensor ops
                # run in the DVE 16-bit 2x mode
                bcst = dblp.tile([32, T], FP32, tag="bcst")
                nc.sync.dma_start(bcst[:], cc_out[64:96, :])
                bcst_bf = dblp.tile([32, T], BF16, tag="bcstbf")
                nc.vector.tensor_copy(bcst_bf[:], bcst[:])
                bc_dram = drp.tile([32, T], BF16, tag="bcdram")
                nc.sync.dma_start(bc_dram[:], bcst_bf[:])
                bc_all = bcp.tile([128, 2 * N, T], BF16, tag="bcall")
                nc.sync.dma_start(
                    bc_all[:],
                    bc_dram[:, :].rearrange("n t -> () n t").broadcast_to(
                        [128, 2 * N, T]
                    ),
                )
                b_all = bc_all[:, 0:N, :]
                c_all = bc_all[:, N : 2 * N, :]

                # ---- dt_proj + softplus ----
                dt_sb = dtp.tile([128, DBLK, T], FP32, tag="dt")
                for m in range(DBLK):
                    pdt = mmp.tile([128, T], FP32, tag="mm")
                    nc.tensor.matmul(
                        pdt[:], wdt_sb[:, ts(m, 128)], dtlow_bf[:], start=True, stop=True
                    )
                    # softplus(v + dtb) as ln(exp(v + dtb) + 1)
                    edt = dblp.tile([128, T], FP32, tag="edt")
                    nc.scalar.activation(
                        edt[:], pdt[:], AF.Exp, bias=dtb_sb[:, m : m + 1]
                    )
                    nc.scalar.activation(dt_sb[:, m, :], edt[:], AF.Ln, bias=1.0)

                dtx = dtxp.tile([128, DBLK, T], BF16, tag="dtx")
                nc.vector.tensor_tensor(dtx[:], dt_sb[:], xc_f[:], op=OP.mult)

                # ---- selective scan over (db, n) slabs ----
                y_sb = yp.tile([128, DBLK, T], FP32, tag="y")
                for db in range(DBLK):
                    for s in range(SLABS):
                        n0 = s * NSLAB
                        da = ap_.tile([128, NSLAB, T], FP32, tag="da")
                        for j in range(NSLAB):
                            nc.scalar.activation(
                                da[:, j, :],
                                dt_sb[:, db, :],
                                AF.Exp,
                                scale=a_sb[:, db, n0 + j : n0 + j + 1],
                            )
                        dbx = bxp.tile([128, NSLAB, T], BF16, tag="dbx")
                        for j in range(NSLAB):
                            nc.vector.tensor_tensor(
                                dbx[:, j, :], dtx[:, db, :], b_all[:, n0 + j, :],
                                op=OP.mult,
                            )
                        h = hp.tile([128, NSLAB, T], BF16, tag="h")
                        for j in range(NSLAB):
                            ci = db * N + n0 + j
                            nc.vector.tensor_tensor_scan(
                                h[:, j, :],
                                da[:, j, :],
                                dbx[:, j, :],
                                initial=carry[:, ci : ci + 1],
                                op0=OP.mult,
                                op1=OP.add,
                            )
                        nc.vector.tensor_copy(
                            carry[:, db * N + n0 : db * N + n0 + NSLAB],
                            h[:, :, T - 1],
                        )
                        hc = hcp.tile([128, NSLAB, T], BF16, tag="hc")
                        nc.vector.tensor_tensor(
                            hc[:], h[:], c_all[:, n0 : n0 + NSLAB, :], op=OP.mult
                        )
                        if s == 0:
                            nc.vector.tensor_reduce(
                                y_sb[:, db, :],
                                hc.rearrange("p n t -> p t n"),
                                axis=mybir.AxisListType.X,
                                op=OP.add,
                            )
                        else:
                            yr = yrp.tile([128, T], FP32, tag="yr")
                            nc.vector.tensor_reduce(
                                yr[:],
                                hc.rearrange("p n t -> p t n"),
                                axis=mybir.AxisListType.X,
                                op=OP.add,
                            )
                            nc.vector.tensor_tensor(
                                y_sb[:, db, :], y_sb[:, db, :], yr[:], op=OP.add
                            )

                # ---- D skip term, gate, out_proj ----
                for db in range(DBLK):
                    nc.vector.scalar_tensor_tensor(
                        y_sb[:, db, :],
                        xc_f[:, db, :],
                        d_sb[:, db : db + 1],
                        y_sb[:, db, :],
                        op0=OP.mult,
                        op1=OP.add,
                    )
                gated = gp.tile([128, DBLK, T], BF16, tag="gated")
                nc.vector.tensor_tensor(gated[:], y_sb[:], zs_sb[:], op=OP.mult)

                out_sb = op_.tile([128, 8, T], BF16, tag="out")
                for m in range(8):
                    po = pout.tile([128, T], FP32, tag="po")
                    for db in range(DBLK):
                        nc.tensor.matmul(
                            po[:],
                            wop_sb[:, db, ts(m, 128)],
                            gated[:, db, :],
                            start=(db == 0),
                            stop=(db == DBLK - 1),
                        )
                    nc.scalar.activation(out_sb[:, m, :], po[:], AF.Copy)
                nc.sync.dma_start(pout_dram[:, :, ds(t0, T)], out_sb[:])

            # ---- ReduceScatter partial outputs across the 4 shard cores ----
            rs_out = gdp.tile([PQ, 8, L], BF16, tag="rsout")
            nc.gpsimd.collective_compute(
                "ReduceScatter",
                OP.add,
                replica_groups=REPLICA_GROUPS,
                ins=[pout_dram[:]],
                outs=[rs_out[:]],
            )
            nc.sync.dma_start(out[:], rs_out[:])

    nc.finalize()
    return nc


_NC_CACHE = {}


def get_nc():
    if "nc" not in _NC_CACHE:
        _NC_CACHE["nc"] = build_nc()
    return _NC_CACHE["nc"]


def make_in_maps(inputs):
    hs = np.asarray(inputs["hidden_states"], np.float32)
    w_in = np.asarray(inputs["in_proj_w"], np.float32)
    conv_w = np.asarray(inputs["conv_w"], np.float32)
    conv_b = np.asarray(inputs["conv_b"], np.float32)
    w_xp = np.asarray(inputs["x_proj_w"], np.float32)
    w_dt = np.asarray(inputs["dt_proj_w"], np.float32)
    b_dt = np.asarray(inputs["dt_proj_b"], np.float32)
    a_log = np.asarray(inputs["A_log"], np.float32)
    d_skip = np.asarray(inputs["D"], np.float32)
    w_op = np.asarray(inputs["out_proj_w"], np.float32)

    a_full = -np.exp(a_log)  # (DI, N)

    in_maps = []
    for c in range(NCORE):
        b, s = divmod(c, NSH)
        d0 = s * DS
        sl = slice(d0, d0 + DS)

        hidT = np.ascontiguousarray(
            hs[b].T.reshape(8, 128, L).transpose(1, 0, 2)[:, :, s * LQ : (s + 1) * LQ]
        ).astype(BF)

        w_cat = np.concatenate([w_in[sl], w_in[DI + d0 : DI + d0 + DS]], 0)  # (1024, DM)
        wxzT = np.ascontiguousarray(
            w_cat.T.reshape(8, 128, 2 * DS).transpose(1, 0, 2)
        ).astype(BF)

        wxpT = np.ascontiguousarray(
            w_xp[:, sl].T.reshape(DBLK, 128, 96).transpose(1, 0, 2)
        ).astype(BF)
        wdtT = np.ascontiguousarray(w_dt[sl].T).astype(BF)  # (64, 512)
        wopT = np.ascontiguousarray(
            w_op[:, sl].T.reshape(DBLK, 128, DM).transpose(1, 0, 2)
        ).astype(BF)

        convw = np.ascontiguousarray(
            conv_w[sl].reshape(DBLK, 128, DC).transpose(1, 0, 2), np.float32
        )
        convb = np.ascontiguousarray(
            conv_b[sl].reshape(DBLK, 128).T, np.float32
        )
        dtb = np.ascontiguousarray(b_dt[sl].reshape(DBLK, 128).T, np.float32)
        dsk = np.ascontiguousarray(d_skip[sl].reshape(DBLK, 128).T, np.float32)
        acol = np.ascontiguousarray(
            a_full[sl].reshape(DBLK, 128, N).transpose(1, 0, 2), np.float32
        )

        consts = np.concatenate(
            [convw.reshape(128, 16), convb, dtb, dsk, acol.reshape(128, 64)], axis=1
        ).astype(np.float32)

        in_maps.append(
            dict(
                hidT=hidT, wxzT=wxzT, wxpT=wxpT, wdtT=wdtT, wopT=wopT,
                consts=np.ascontiguousarray(consts),
            )
        )
    return in_maps


def gather_output(results):
    outs = []
    for b in range(B):
        # core (b, s) holds partition rows [s*PQ, (s+1)*PQ) of the reduced
        # (128, 8, L) output (ReduceScatter splits the flat buffer into 4
        # contiguous chunks, rank s gets chunk s)
        acc = np.concatenate(
            [np.asarray(results[b * NSH + s]["out"], np.float32) for s in range(NSH)],
            axis=0,
        )
        # (128, 8, L) [p, m, t] -> (1024, L) -> (L, 1024)
        full_t = acc.transpose(1, 0, 2).reshape(DM, L)
        outs.append(full_t.T)
    return np.stack(outs).astype(np.float32)


def run_on_hw(inputs, trace=False, **kwargs):
    nc = get_nc()
    in_maps = make_in_maps(inputs)
    res = run_bass_kernel_spmd(
        nc, in_maps, core_ids=list(range(NCORE)), trace=trace, **kwargs
    )
    return res


def kernel(**inputs):
    res = run_on_hw(inputs, trace=False)
    return gather_output(res.results)

